# revision 21
# baseline (speedup 1.0000x reference)
"""AttnBlock3D Trainium2 kernel (8-core frame-parallel).

Math (per reference):
  hn = GroupNorm32(x) * gamma + beta          # stats over (c/32, t, h, w) -> global over frames
  q/k/v = hn @ w{q,k,v} + b{q,k,v}            # per-frame, per-position linear over channels
  attn  = softmax(q @ k.T / sqrt(c))          # per frame, positions hw=4096
  o     = attn @ v @ wp + bp
  out   = x + o

Distribution: one frame (b*t = 8) per NeuronCore. GroupNorm stats need a
cross-frame reduction: each core computes per-channel sum/sumsq over its
frame, a 4KB AllReduce combines them, then everything else is local.
Group-stat math (16-channel segment sums, group->channel broadcast) runs on
the PE via tiny indicator-matrix matmuls to avoid slow 1-partition DVE ops.

On-chip layouts (SBUF partitions x free):
  XN  [c=512 (4x128), pos=4096] bf16     normalized activations, transposed
  KT  [c_out (4x128), pos=4096] bf16     k transposed
  V   [pos (32x128), c_out=512] bf16     v natural
  per q-block (512 positions), flash-pipelined over k-chunks j:
    S_j psum [128, 512] -> exp -> P_j bf16 (few rotating slots)
    d  psum [1,512] += ones.T @ P_j   (softmax denominators on PE)
    O  psum [128c, 512] += V_j.T @ P_j
    r = recip(bcast(d)); OT = O * r; out = wp.T @ OT + bp' + x (f32 residual)
  Biases: q,k as per-partition ACT bias; v,p folded: bp' = wp.T @ bv + bp.
"""

import sys

sys.path.insert(0, "/opt/trn_rl_repo")

import numpy as np

import concourse.bacc as bacc
import concourse.bass as bass
import concourse.mybir as mybir
import concourse.tile as tile
from concourse.bass_utils import run_bass_kernel_spmd

N_CORES = 8
C = 512  # channels
S = 4096  # positions per frame (h*w)
G = 32  # groups
CPG = C // G  # 16 channels per group
PCH = C // 128  # 4 channel chunks of 128 partitions
KCH = S // 128  # 32 position chunks of 128
QB = 512  # q-block size
NQB = S // QB  # 8 q blocks
NTOT = CPG * 8 * S  # group-norm element count per group (global over 8 frames)
EPS = 1e-6
SCALE = float(C) ** -0.5

F32 = mybir.dt.float32
BF16 = mybir.dt.bfloat16
AF = mybir.ActivationFunctionType
ALU = mybir.AluOpType
AX = mybir.AxisListType

_NC_CACHE = {}
DEBUG = False


def xs2_dst8(nc, dst):
    return dst


def build_nc():
    nc = bacc.Bacc("TRN2", target_bir_lowering=False, debug=False, num_devices=N_CORES)

    x_in = nc.dram_tensor("x", [C, S], F32, kind="ExternalInput")
    gamma_in = nc.dram_tensor("gamma", [C], F32, kind="ExternalInput")
    beta_in = nc.dram_tensor("beta", [C], F32, kind="ExternalInput")
    w_in = {}
    b_in = {}
    for nm in ("wq", "wk", "wv", "wp"):
        w_in[nm] = nc.dram_tensor(nm, [C, C], F32, kind="ExternalInput")
    for nm in ("bq", "bk", "bv", "bp"):
        b_in[nm] = nc.dram_tensor(nm, [C], F32, kind="ExternalInput")
    out_d = nc.dram_tensor("out", [C, S], F32, kind="ExternalOutput")
    dbg_d = nc.dram_tensor("dbg", [128, 64], F32, kind="ExternalOutput") if DEBUG else None

    with tile.TileContext(nc) as tc:
        with (
            tc.tile_pool(name="persist", bufs=1) as pp,
            tc.tile_pool(name="psum", bufs=1, space="PSUM") as psp,
            tc.tile_pool(name="dram", bufs=1, space="DRAM") as dram,
        ):
            # ---- persistent SBUF ----
            FP8 = mybir.dt.float8e4
            # fp8 pair tiles for DoubleRow matmuls: [:, parity, :]
            XN2 = [pp.tile([128, 2, S], FP8, name=f"XN2_{cc}") for cc in range(2)]
            KT2 = [pp.tile([128, 2, S], FP8, name=f"KT2_{cc}") for cc in range(2)]
            V2 = [pp.tile([128, 2, C], FP8, name=f"V2_{jj}") for jj in range(KCH // 2)]
            W2 = {
                nm: [pp.tile([128, 2, C], FP8, name=f"{nm}2_{cc}") for cc in range(2)]
                for nm in ("wq", "wk", "wv")
            }
            Wp = [pp.tile([128, C], BF16, name=f"wp_{p}") for p in range(PCH)]
            Wp2 = [pp.tile([128, 2, C], FP8, name=f"wp2_{cc}") for cc in range(2)]
            bq_p = [pp.tile([128, 1], F32, name=f"bqp{p}") for p in range(PCH)]
            bk_p = [pp.tile([128, 1], F32, name=f"bkp{p}") for p in range(PCH)]
            bv_bf = [pp.tile([128, 1], BF16, name=f"bvb{p}") for p in range(PCH)]
            bpp_p = [pp.tile([128, 1], F32, name=f"bppp{p}") for p in range(PCH)]
            sc_p = [pp.tile([128, 1], F32, name=f"scp{p}") for p in range(PCH)]
            bc_p = [pp.tile([128, 1], F32, name=f"bcp{p}") for p in range(PCH)]
            ones2 = pp.tile([128, 2, 16], FP8, name="ones2")
            nc.vector.memset(ones2[:], 1.0)

            F32R = mybir.dt.float32r
            ones_row_f = pp.tile([1, 128], F32, name="ones_row_f")
            ones_row = pp.tile([1, 128], F32R, name="ones_row")
            nc.vector.memset(ones_row_f[:], 1.0)
            nc.vector.tensor_copy(ones_row[:], ones_row_f[:])

            # ---- prologue pool (released before attention main loop) ----
            prolog_cm = tc.tile_pool(name="prolog", bufs=1)
            pl = prolog_cm.__enter__()

            # ---- pass 1 first: stream x (critical path), sum & sumsq ----
            sum_t = [pl.tile([128, 1], F32, name=f"sum{p}") for p in range(PCH)]
            ssq_t = [pl.tile([128, 1], F32, name=f"ssq{p}") for p in range(PCH)]
            for p in range(PCH):
                xs = pl.tile([128, S], F32, name="xs", tag="xstream", bufs=2)
                nc.sync.dma_start(xs[:], x_in[p * 128 : (p + 1) * 128, :])
                nc.vector.reduce_sum(sum_t[p][:], xs[:], axis=AX.X)
                junk = pl.tile([128, S], BF16, name="junk", tag="junk", bufs=1)
                nc.scalar.activation(junk[:], xs[:], AF.Square, accum_out=ssq_t[p][:])

            # pack stats -> [128, 8]: cols 0-3 sums, 4-7 sumsq; AllReduce
            stats8 = pl.tile([128, 8], F32, name="stats8")
            for p in range(PCH):
                nc.vector.tensor_copy(stats8[:, p : p + 1], sum_t[p][:])
                nc.vector.tensor_copy(stats8[:, 4 + p : 5 + p], ssq_t[p][:])
            cc_in = dram.tile([128, 8], F32, name="cc_in")
            cc_out = dram.tile([128, 8], F32, name="cc_out", addr_space="Shared")
            nc.sync.dma_start(cc_in[:], stats8[:])
            nc.gpsimd.collective_compute(
                "AllReduce",
                ALU.add,
                replica_groups=[list(range(N_CORES))],
                ins=[cc_in.opt()],
                outs=[cc_out.opt()],
            )

            # ---- weight load + cast (streamed); wp first (bp' needs it) ----
            for nm in ("wp", "wq", "wk", "wv"):
                for p in range(PCH):
                    wstg = pl.tile([128, C], F32, name="wstg", tag="wstg", bufs=2)
                    nc.gpsimd.dma_start(wstg[:], w_in[nm][p * 128 : (p + 1) * 128, :])
                    if nm == "wp":
                        nc.vector.tensor_copy(Wp[p][:], wstg[:])
                        nc.vector.tensor_copy(Wp2[p // 2][:, p % 2, :], wstg[:])
                    else:
                        nc.vector.tensor_copy(W2[nm][p // 2][:, p % 2, :], wstg[:])

            # ---- small loads (off critical path) ----
            for p in range(PCH):
                nc.gpsimd.dma_start(bq_p[p][:], b_in["bq"][p * 128 : (p + 1) * 128, None])
                nc.gpsimd.dma_start(bk_p[p][:], b_in["bk"][p * 128 : (p + 1) * 128, None])
            bv_st = [pl.tile([128, 1], F32, name=f"bvst{p}") for p in range(PCH)]
            bp_p = [pl.tile([128, 1], F32, name=f"bpst{p}") for p in range(PCH)]
            gam4 = pl.tile([128, 4], F32, name="gam4")
            bet4 = pl.tile([128, 4], F32, name="bet4")
            # channel c=(part,p): DRAM idx 128p+part -> [128 part, 4 p] strided
            nc.gpsimd.dma_start(gam4[:], gamma_in[:].rearrange("(p a) -> a p", p=4, a=128))
            nc.gpsimd.dma_start(bet4[:], beta_in[:].rearrange("(p a) -> a p", p=4, a=128))
            for p in range(PCH):
                sl = slice(p * 128, (p + 1) * 128)
                nc.gpsimd.dma_start(bv_st[p][:], b_in["bv"][sl, None])
                nc.gpsimd.dma_start(bp_p[p][:], b_in["bp"][sl, None])
                nc.vector.tensor_copy(bv_bf[p][:], bv_st[p][:])

            # indicator matrices for group-segment sums / broadcasts
            ind_np = np.zeros((128, 8), np.float32)  # [part, gl] = part//16==gl
            for gl in range(8):
                ind_np[16 * gl : 16 * (gl + 1), gl] = 1.0
            ind_d = nc.inline_tensor(ind_np, name="ind_const")
            indt_d = nc.inline_tensor(np.ascontiguousarray(ind_np.T), name="indt_const")
            IND = pl.tile([128, 8], F32, name="IND")
            INDT = pl.tile([8, 128], F32, name="INDT")
            nc.sync.dma_start(IND[:], ind_d[:, :])
            nc.sync.dma_start(INDT[:], indt_d[:, :])

            # ---- bp' = wp.T @ bv + bp via N=1 matmuls ----
            for m in range(PCH):
                ps_bp = psp.tile([128, 1], F32, name="ps_bp", tag="ps_d", bufs=1)
                for ci in range(PCH):
                    nc.tensor.matmul(
                        ps_bp[:],
                        Wp[ci][:, m * 128 : (m + 1) * 128],
                        bv_bf[ci][:],
                        start=(ci == 0),
                        stop=(ci == PCH - 1),
                    )
                nc.vector.tensor_tensor(bpp_p[m][:], ps_bp[:], bp_p[m][:], op=ALU.add)

            # ---- post-collective: group stats on PE ----
            stats_g = pl.tile([128, 8], F32, name="stats_g")
            nc.sync.dma_start(stats_g[:], cc_out[:])
            ps_g = psp.tile([8, 8], F32, name="ps_g", tag="ps_d", bufs=1)
            # out[gl, j] = sum_part IND[part, gl] * stats_g[part, j]
            nc.tensor.matmul(ps_g[:], IND[:], stats_g[:], start=True, stop=True)
            gs8 = pl.tile([8, 8], F32, name="gs8")
            nc.vector.tensor_copy(gs8[:], ps_g[:])
            # per-group mean/rstd on 8 partitions x 4 chunks
            invN = 1.0 / float(NTOT)
            mean8 = pl.tile([8, 4], F32, name="mean8")
            var8 = pl.tile([8, 4], F32, name="var8")
            rstd8 = pl.tile([8, 4], F32, name="rstd8")
            eps8 = pl.tile([8, 1], F32, name="eps8")
            nc.vector.memset(eps8[:], EPS)
            nc.vector.tensor_scalar_mul(mean8[:], gs8[:, 0:4], invN)
            nc.vector.tensor_scalar_mul(var8[:], gs8[:, 4:8], invN)
            nc.vector.tensor_tensor(rstd8[:], mean8[:], mean8[:], op=ALU.mult)
            nc.vector.tensor_tensor(var8[:], var8[:], rstd8[:], op=ALU.subtract)
            nc.scalar.activation(var8[:], var8[:], AF.Sqrt, bias=eps8[:])
            nc.vector.reciprocal(rstd8[:], var8[:])
            # pack [rstd | mean] and broadcast groups -> 128 partitions via PE
            rm8 = pl.tile([8, 8], F32, name="rm8")
            nc.vector.tensor_copy(rm8[:, 0:4], rstd8[:])
            nc.vector.tensor_copy(rm8[:, 4:8], mean8[:])
            ps_e = psp.tile([128, 8], F32, name="ps_e", tag="ps_d", bufs=1)
            nc.tensor.matmul(ps_e[:], INDT[:], rm8[:], start=True, stop=True)
            # sc = gamma * rstd; bc = beta - mean * sc   (all chunks at once)
            sc4 = pl.tile([128, 4], F32, name="sc4")
            bc4 = pl.tile([128, 4], F32, name="bc4")
            nc.vector.tensor_tensor(sc4[:], gam4[:], ps_e[:, 0:4], op=ALU.mult)
            nc.vector.tensor_tensor(bc4[:], ps_e[:, 4:8], sc4[:], op=ALU.mult)
            nc.vector.tensor_tensor(bc4[:], bet4[:], bc4[:], op=ALU.subtract)

            if DEBUG:
                dbg_t = pl.tile([128, 64], F32, name="dbg_t")
                nc.vector.memset(dbg_t[:], 0.0)
                nc.vector.tensor_copy(dbg_t[:, 0:8], stats_g[:])
                nc.vector.tensor_copy(dbg_t[0:8, 8:16], gs8[:])
                nc.vector.tensor_copy(dbg_t[0:8, 16:20], mean8[:])
                nc.vector.tensor_copy(dbg_t[0:8, 20:24], var8[:])
                nc.vector.tensor_copy(dbg_t[0:8, 24:28], rstd8[:])
                nc.vector.tensor_copy(dbg_t[:, 28:36], ps_e[:])
                for p in range(PCH):
                    nc.vector.tensor_copy(dbg_t[:, 36 + p : 37 + p], sc_p[p][:])
                    nc.vector.tensor_copy(dbg_t[:, 40 + p : 41 + p], bc_p[p][:])
                    nc.vector.tensor_copy(dbg_t[:, 44 + p : 45 + p], sum_t[p][:])
                    nc.vector.tensor_copy(dbg_t[:, 48 + p : 49 + p], ssq_t[p][:])
                nc.vector.tensor_copy(dbg_t[:, 52:60], IND[:])
                nc.sync.dma_start(dbg_d[:, :], dbg_t[:])

            # ---- pass 2: re-stream x, normalize -> XN2 fp8. n-outer order so
            # the first K^T matmuls (which need all 4 chunks of one n-slice)
            # can start after ~1/8 of the pass ----
            xs2_t = []
            for p in range(PCH):
                xs2 = pl.tile([128, S], F32, name="xs2", tag="xs2", bufs=4)
                nc.sync.dma_start(xs2[:], x_in[p * 128 : (p + 1) * 128, :])
                xs2_t.append(xs2)
            for n in range(NQB):
                for p in range(PCH):
                    nsl = slice(n * QB, (n + 1) * QB)
                    dst = XN2[p // 2][:, p % 2, nsl]
                    if (p + n) % 2 == 0:
                        nc.vector.tensor_scalar(
                            dst, xs2_t[p][:, nsl], sc4[:, p : p + 1], bc4[:, p : p + 1],
                            op0=ALU.mult, op1=ALU.add,
                        )
                    else:
                        nc.scalar.activation(
                            dst, xs2_t[p][:, nsl], AF.Identity,
                            bias=bc4[:, p : p + 1], scale=sc4[:, p : p + 1],
                        )

            prolog_cm.__exit__(None, None, None)

            # ---- main-loop pool ----
            mainloop_cm = tc.tile_pool(name="mainloop", bufs=1)
            ml = mainloop_cm.__enter__()

            # ---- K^T (bias via ACT) and V via DoubleRow on XN2 pairs ----
            DR = mybir.MatmulPerfMode.DoubleRow
            for n in range(NQB):
                for m in range(PCH):
                    ps_k = psp.tile([128, QB], F32, name="ps_k", tag="ps_s", bufs=2)
                    for cc in range(2):
                        nc.tensor.matmul(
                            ps_k[:],
                            W2["wk"][cc][:, :, m * 128 : (m + 1) * 128],
                            XN2[cc][:, :, n * QB : (n + 1) * QB],
                            perf_mode=DR,
                            start=(cc == 0),
                            stop=(cc == 1),
                        )
                    nc.scalar.activation(
                        KT2[m // 2][:, m % 2, n * QB : (n + 1) * QB],
                        ps_k[:],
                        AF.Identity,
                        bias=bk_p[m][:],
                    )
            for j in range(KCH):
                ps_v = psp.tile([128, C], F32, name="ps_v", tag="ps_s", bufs=2)
                for cc in range(2):
                    nc.tensor.matmul(
                        ps_v[:],
                        XN2[cc][:, :, j * 128 : (j + 1) * 128],
                        W2["wv"][cc][:, :, :],
                        perf_mode=DR,
                        start=(cc == 0),
                        stop=(cc == 1),
                    )
                nc.vector.tensor_copy(V2[j // 2][:, j % 2, :], ps_v[:])

            # ---- attention main loop over q-blocks ----
            def emit_qt(qb, m, QT):
                ps_q = psp.tile([128, QB], F32, name="ps_q", tag="ps_q", bufs=1)
                for cc in range(2):
                    nc.tensor.matmul(
                        ps_q[:],
                        W2["wq"][cc][:, :, m * 128 : (m + 1) * 128],
                        XN2[cc][:, :, qb * QB : (qb + 1) * QB],
                        perf_mode=mybir.MatmulPerfMode.DoubleRow,
                        start=(cc == 0),
                        stop=(cc == 1),
                    )
                nc.scalar.activation(
                    QT[m // 2][:, m % 2, :], ps_q[:], AF.Identity, bias=bq_p[m][:]
                )

            def make_qt():
                return [
                    ml.tile([128, 2, QB], FP8, name=f"QT{cc}", tag=f"QT{cc}", bufs=2)
                    for cc in range(2)
                ]

            QT_cur = make_qt()
            for m in range(PCH):
                emit_qt(0, m, QT_cur)

            def emit_s(j, QT, P2pair):
                """scores S^T[j] via DoubleRow fp8 -> exp -> P2 half."""
                ps_s = psp.tile([128, QB], F32, name="ps_s", tag="ps_s", bufs=2)
                for cc in range(2):
                    nc.tensor.matmul(
                        ps_s[:],
                        KT2[cc][:, :, j * 128 : (j + 1) * 128],
                        QT[cc][:],
                        perf_mode=mybir.MatmulPerfMode.DoubleRow,
                        start=(cc == 0),
                        stop=(cc == 1),
                    )
                nc.scalar.activation(
                    P2pair[:, j % 2, :], ps_s[:], AF.Exp, scale=SCALE
                )

            NJJ = KCH // 2  # 16 pairs
            for qb in range(NQB):
                QT_next = None
                # software-pipelined over PAIRS: s one pair ahead, then
                # d_jj + PV_jj consume pair jj (DoubleRow fp8)
                ps_dd = psp.tile([1, QB], F32, name="ps_dd", tag="ps_d", bufs=1)
                ps_o = [
                    psp.tile([128, QB], F32, name=f"ps_o{mc}", tag=f"ps_o{mc}", bufs=1)
                    for mc in range(PCH)
                ]

                def make_pair():
                    return ml.tile([128, 2, QB], FP8, name="P2", tag="P2", bufs=4)

                P2s = [None] * NJJ
                P2s[0] = make_pair()
                emit_s(0, QT_cur, P2s[0])
                emit_s(1, QT_cur, P2s[0])
                P2s[1] = make_pair()
                emit_s(2, QT_cur, P2s[1])
                emit_s(3, QT_cur, P2s[1])
                for jj in range(NJJ):
                    if jj + 2 < NJJ:
                        P2s[jj + 2] = make_pair()
                        emit_s(2 * jj + 4, QT_cur, P2s[jj + 2])
                        emit_s(2 * jj + 5, QT_cur, P2s[jj + 2])
                    nc.tensor.matmul(
                        ps_dd[:],
                        ones2[:, :, 0:1],
                        P2s[jj][:],
                        perf_mode=mybir.MatmulPerfMode.DoubleRow,
                        start=(jj == 0),
                        stop=(jj == NJJ - 1),
                    )
                    for mc in range(PCH):
                        nc.tensor.matmul(
                            ps_o[mc][:],
                            V2[jj][:, :, mc * 128 : (mc + 1) * 128],
                            P2s[jj][:],
                            perf_mode=mybir.MatmulPerfMode.DoubleRow,
                            start=(jj == 0),
                            stop=(jj == NJJ - 1),
                        )
                    P2s[jj] = None
                    # interleave next block's q^T generation into the PV stream
                    if jj % 4 == 1 and qb + 1 < NQB:
                        if QT_next is None:
                            QT_next = make_qt()
                        emit_qt(qb + 1, (jj - 1) // 4, QT_next)

                # raw attention sums -> bf16 via ACT (fast, off DVE); the
                # 1/d column scaling commutes with the wp matmul and is
                # applied in the epilogue instead.
                OT2 = [
                    ml.tile([128, 2, QB], FP8, name=f"OT2_{cc}", tag=f"OT2_{cc}", bufs=1)
                    for cc in range(2)
                ]
                for mc in range(PCH):
                    if mc % 2 == 0:
                        nc.scalar.copy(OT2[mc // 2][:, mc % 2, :], ps_o[mc][:])
                    else:
                        nc.vector.tensor_copy(OT2[mc // 2][:, mc % 2, :], ps_o[mc][:])

                # denominators -> r broadcast (fully overlapped with wp MMs):
                # ACT copy psum->sbuf, PE rank-1 f32r broadcast, wide DVE recip
                d_sb = ml.tile([1, QB], mybir.dt.float32r, name="d_sb", tag="d_sb", bufs=2)
                r_bc = ml.tile([128, QB], F32, name="r_bc", tag="r_bc", bufs=2)
                nc.scalar.copy(d_sb[:], ps_dd[:])
                ps_r = psp.tile([128, QB], F32, name="ps_r", tag="ps_q", bufs=1)
                nc.tensor.matmul(ps_r[:], ones_row[:], d_sb[:], start=True, stop=True)
                nc.vector.reciprocal(r_bc[:], ps_r[:])

                # project by wp; epilogue: scale by r, add bp' and residual x
                q0 = qb * QB
                for m in range(PCH):
                    ps_f = psp.tile([128, QB], F32, name="ps_f", tag=f"ps_o{m}", bufs=1)
                    for cc in range(2):
                        nc.tensor.matmul(
                            ps_f[:],
                            Wp2[cc][:, :, m * 128 : (m + 1) * 128],
                            OT2[cc][:],
                            perf_mode=mybir.MatmulPerfMode.DoubleRow,
                            start=(cc == 0),
                            stop=(cc == 1),
                        )
                    xr = ml.tile([128, QB], F32, name="xr", tag="xr", bufs=4)
                    nc.sync.dma_start(xr[:], x_in[m * 128 : (m + 1) * 128, q0 : q0 + QB])
                    on_ = ml.tile([128, QB], F32, name="on", tag="on", bufs=4)
                    os_ = ml.tile([128, QB], F32, name="os", tag="os", bufs=4)
                    nc.vector.tensor_tensor(on_[:], ps_f[:], r_bc[:], op=ALU.mult)
                    nc.vector.scalar_tensor_tensor(
                        os_[:], on_[:], bpp_p[m][:], xr[:], op0=ALU.add, op1=ALU.add
                    )
                    nc.sync.dma_start(
                        out_d[m * 128 : (m + 1) * 128, q0 : q0 + QB], os_[:]
                    )
                if QT_next is not None:
                    QT_cur = QT_next

            mainloop_cm.__exit__(None, None, None)

    nc.compile()
    return nc


def _get_nc():
    if "nc" not in _NC_CACHE:
        _NC_CACHE["nc"] = build_nc()
    return _NC_CACHE["nc"]


def kernel(x, gamma, beta, wq, bq, wk, bk, wv, bv, wp, bp, **_unused):
    x = np.asarray(x, np.float32)
    b, c, t, h, w = x.shape
    assert (b, c, t, h, w) == (1, C, 8, 64, 64)
    nc = _get_nc()

    shared = {
        "gamma": np.ascontiguousarray(np.asarray(gamma, np.float32)),
        "beta": np.ascontiguousarray(np.asarray(beta, np.float32)),
        "wq": np.ascontiguousarray(np.asarray(wq, np.float32)),
        "bq": np.ascontiguousarray(np.asarray(bq, np.float32)),
        "wk": np.ascontiguousarray(np.asarray(wk, np.float32)),
        "bk": np.ascontiguousarray(np.asarray(bk, np.float32)),
        "wv": np.ascontiguousarray(np.asarray(wv, np.float32)),
        "bv": np.ascontiguousarray(np.asarray(bv, np.float32)),
        "wp": np.ascontiguousarray(np.asarray(wp, np.float32)),
        "bp": np.ascontiguousarray(np.asarray(bp, np.float32)),
    }
    in_maps = []
    for ti in range(t):
        frame = np.ascontiguousarray(x[0, :, ti, :, :].reshape(C, S))
        in_maps.append({"x": frame, **shared})

    res = run_bass_kernel_spmd(nc, in_maps, core_ids=list(range(N_CORES)))

    out = np.empty((1, C, t, h, w), np.float32)
    for ti in range(t):
        out[0, :, ti, :, :] = res.results[ti]["out"].reshape(C, h, w)
    return out


# revision 22
# speedup vs baseline: 1.0585x; 1.0585x over previous
"""AttnBlock3D Trainium2 kernel (8-core frame-parallel).

Math (per reference):
  hn = GroupNorm32(x) * gamma + beta          # stats over (c/32, t, h, w) -> global over frames
  q/k/v = hn @ w{q,k,v} + b{q,k,v}            # per-frame, per-position linear over channels
  attn  = softmax(q @ k.T / sqrt(c))          # per frame, positions hw=4096
  o     = attn @ v @ wp + bp
  out   = x + o

Distribution: one frame (b*t = 8) per NeuronCore. GroupNorm stats need a
cross-frame reduction: each core computes per-channel sum/sumsq over its
frame, a 4KB AllReduce combines them, then everything else is local.
Group-stat math (16-channel segment sums, group->channel broadcast) runs on
the PE via tiny indicator-matrix matmuls to avoid slow 1-partition DVE ops.

On-chip layouts (SBUF partitions x free):
  XN  [c=512 (4x128), pos=4096] bf16     normalized activations, transposed
  KT  [c_out (4x128), pos=4096] bf16     k transposed
  V   [pos (32x128), c_out=512] bf16     v natural
  per q-block (512 positions), flash-pipelined over k-chunks j:
    S_j psum [128, 512] -> exp -> P_j bf16 (few rotating slots)
    d  psum [1,512] += ones.T @ P_j   (softmax denominators on PE)
    O  psum [128c, 512] += V_j.T @ P_j
    r = recip(bcast(d)); OT = O * r; out = wp.T @ OT + bp' + x (f32 residual)
  Biases: q,k as per-partition ACT bias; v,p folded: bp' = wp.T @ bv + bp.
"""

import sys

sys.path.insert(0, "/opt/trn_rl_repo")

import numpy as np

import concourse.bacc as bacc
import concourse.bass as bass
import concourse.mybir as mybir
import concourse.tile as tile
from concourse.bass_utils import run_bass_kernel_spmd

N_CORES = 8
C = 512  # channels
S = 4096  # positions per frame (h*w)
G = 32  # groups
CPG = C // G  # 16 channels per group
PCH = C // 128  # 4 channel chunks of 128 partitions
KCH = S // 128  # 32 position chunks of 128
QB = 512  # q-block size
NQB = S // QB  # 8 q blocks
NTOT = CPG * 8 * S  # group-norm element count per group (global over 8 frames)
EPS = 1e-6
SCALE = float(C) ** -0.5

F32 = mybir.dt.float32
BF16 = mybir.dt.bfloat16
AF = mybir.ActivationFunctionType
ALU = mybir.AluOpType
AX = mybir.AxisListType

_NC_CACHE = {}
DEBUG = False


def xs2_dst8(nc, dst):
    return dst


def build_nc():
    nc = bacc.Bacc("TRN2", target_bir_lowering=False, debug=False, num_devices=N_CORES)

    x_in = nc.dram_tensor("x", [C, S], F32, kind="ExternalInput")
    gamma_in = nc.dram_tensor("gamma", [C], F32, kind="ExternalInput")
    beta_in = nc.dram_tensor("beta", [C], F32, kind="ExternalInput")
    w_in = {}
    b_in = {}
    for nm in ("wq", "wk", "wv", "wp"):
        w_in[nm] = nc.dram_tensor(nm, [C, C], F32, kind="ExternalInput")
    for nm in ("bq", "bk", "bv", "bp"):
        b_in[nm] = nc.dram_tensor(nm, [C], F32, kind="ExternalInput")
    out_d = nc.dram_tensor("out", [C, S], F32, kind="ExternalOutput")
    dbg_d = nc.dram_tensor("dbg", [128, 64], F32, kind="ExternalOutput") if DEBUG else None

    with tile.TileContext(nc) as tc:
        with (
            tc.tile_pool(name="persist", bufs=1) as pp,
            tc.tile_pool(name="psum", bufs=1, space="PSUM") as psp,
            tc.tile_pool(name="dram", bufs=1, space="DRAM") as dram,
        ):
            # ---- persistent SBUF ----
            FP8 = mybir.dt.float8e4
            # fp8 pair tiles for DoubleRow matmuls: [:, parity, :]
            XN2 = [pp.tile([128, 2, S], FP8, name=f"XN2_{cc}") for cc in range(2)]
            KT2 = [pp.tile([128, 2, S], FP8, name=f"KT2_{cc}") for cc in range(2)]
            V2 = [pp.tile([128, 2, C], FP8, name=f"V2_{jj}") for jj in range(KCH // 2)]
            W2 = {
                nm: [pp.tile([128, 2, C], FP8, name=f"{nm}2_{cc}") for cc in range(2)]
                for nm in ("wq", "wk", "wv")
            }
            Wp = [pp.tile([128, C], BF16, name=f"wp_{p}") for p in range(PCH)]
            Wp2 = [pp.tile([128, 2, C], FP8, name=f"wp2_{cc}") for cc in range(2)]
            bq_p = [pp.tile([128, 1], F32, name=f"bqp{p}") for p in range(PCH)]
            bk_p = [pp.tile([128, 1], F32, name=f"bkp{p}") for p in range(PCH)]
            bv_bf = [pp.tile([128, 1], BF16, name=f"bvb{p}") for p in range(PCH)]
            bpp_p = [pp.tile([128, 1], F32, name=f"bppp{p}") for p in range(PCH)]
            sc_p = [pp.tile([128, 1], F32, name=f"scp{p}") for p in range(PCH)]
            bc_p = [pp.tile([128, 1], F32, name=f"bcp{p}") for p in range(PCH)]
            ones2 = pp.tile([128, 2, 16], FP8, name="ones2")
            nc.vector.memset(ones2[:], 1.0)

            F32R = mybir.dt.float32r
            ones_row_f = pp.tile([1, 128], F32, name="ones_row_f")
            ones_row = pp.tile([1, 128], F32R, name="ones_row")
            nc.vector.memset(ones_row_f[:], 1.0)
            nc.vector.tensor_copy(ones_row[:], ones_row_f[:])

            # ---- prologue pool (released before attention main loop) ----
            prolog_cm = tc.tile_pool(name="prolog", bufs=1)
            pl = prolog_cm.__enter__()

            # ---- pass 1 first: stream x (critical path), sum & sumsq ----
            sum_t = [pl.tile([128, 1], F32, name=f"sum{p}") for p in range(PCH)]
            ssq_t = [pl.tile([128, 1], F32, name=f"ssq{p}") for p in range(PCH)]
            for p in range(PCH):
                xs = pl.tile([128, S], F32, name="xs", tag="xstream", bufs=2)
                nc.sync.dma_start(xs[:], x_in[p * 128 : (p + 1) * 128, :])
                nc.vector.reduce_sum(sum_t[p][:], xs[:], axis=AX.X)
                junk = pl.tile([128, S], BF16, name="junk", tag="junk", bufs=1)
                nc.scalar.activation(junk[:], xs[:], AF.Square, accum_out=ssq_t[p][:])

            # pack stats -> [128, 8]: cols 0-3 sums, 4-7 sumsq; AllReduce
            stats8 = pl.tile([128, 8], F32, name="stats8")
            for p in range(PCH):
                nc.vector.tensor_copy(stats8[:, p : p + 1], sum_t[p][:])
                nc.vector.tensor_copy(stats8[:, 4 + p : 5 + p], ssq_t[p][:])
            cc_in = dram.tile([128, 8], F32, name="cc_in")
            cc_out = dram.tile([128, 8], F32, name="cc_out", addr_space="Shared")
            nc.sync.dma_start(cc_in[:], stats8[:])
            nc.gpsimd.collective_compute(
                "AllReduce",
                ALU.add,
                replica_groups=[list(range(N_CORES))],
                ins=[cc_in.opt()],
                outs=[cc_out.opt()],
            )

            # ---- weight load + cast (streamed); wp first (bp' needs it) ----
            for nm in ("wp", "wq", "wk", "wv"):
                for p in range(PCH):
                    wstg = pl.tile([128, C], F32, name="wstg", tag="wstg", bufs=2)
                    nc.sync.dma_start(wstg[:], w_in[nm][p * 128 : (p + 1) * 128, :])
                    if nm == "wp":
                        nc.vector.tensor_copy(Wp[p][:], wstg[:])
                        nc.vector.tensor_copy(Wp2[p // 2][:, p % 2, :], wstg[:])
                    else:
                        nc.vector.tensor_copy(W2[nm][p // 2][:, p % 2, :], wstg[:])

            # ---- small loads (off critical path) ----
            for p in range(PCH):
                nc.sync.dma_start(bq_p[p][:], b_in["bq"][p * 128 : (p + 1) * 128, None])
                nc.sync.dma_start(bk_p[p][:], b_in["bk"][p * 128 : (p + 1) * 128, None])
            bv_st = [pl.tile([128, 1], F32, name=f"bvst{p}") for p in range(PCH)]
            bp_p = [pl.tile([128, 1], F32, name=f"bpst{p}") for p in range(PCH)]
            gam4 = pl.tile([128, 4], F32, name="gam4")
            bet4 = pl.tile([128, 4], F32, name="bet4")
            # channel c=(part,p): DRAM idx 128p+part -> [128 part, 4 p] strided
            nc.sync.dma_start(gam4[:], gamma_in[:].rearrange("(p a) -> a p", p=4, a=128))
            nc.sync.dma_start(bet4[:], beta_in[:].rearrange("(p a) -> a p", p=4, a=128))
            for p in range(PCH):
                sl = slice(p * 128, (p + 1) * 128)
                nc.sync.dma_start(bv_st[p][:], b_in["bv"][sl, None])
                nc.sync.dma_start(bp_p[p][:], b_in["bp"][sl, None])
                nc.vector.tensor_copy(bv_bf[p][:], bv_st[p][:])

            # indicator matrices for group-segment sums / broadcasts
            ind_np = np.zeros((128, 8), np.float32)  # [part, gl] = part//16==gl
            for gl in range(8):
                ind_np[16 * gl : 16 * (gl + 1), gl] = 1.0
            ind_d = nc.inline_tensor(ind_np, name="ind_const")
            indt_d = nc.inline_tensor(np.ascontiguousarray(ind_np.T), name="indt_const")
            IND = pl.tile([128, 8], F32, name="IND")
            INDT = pl.tile([8, 128], F32, name="INDT")
            nc.sync.dma_start(IND[:], ind_d[:, :])
            nc.sync.dma_start(INDT[:], indt_d[:, :])

            # ---- bp' = wp.T @ bv + bp via N=1 matmuls ----
            for m in range(PCH):
                ps_bp = psp.tile([128, 1], F32, name="ps_bp", tag="ps_d", bufs=1)
                for ci in range(PCH):
                    nc.tensor.matmul(
                        ps_bp[:],
                        Wp[ci][:, m * 128 : (m + 1) * 128],
                        bv_bf[ci][:],
                        start=(ci == 0),
                        stop=(ci == PCH - 1),
                    )
                nc.vector.tensor_tensor(bpp_p[m][:], ps_bp[:], bp_p[m][:], op=ALU.add)

            # ---- post-collective: group stats on PE ----
            stats_g = pl.tile([128, 8], F32, name="stats_g")
            nc.sync.dma_start(stats_g[:], cc_out[:])
            ps_g = psp.tile([8, 8], F32, name="ps_g", tag="ps_d", bufs=1)
            # out[gl, j] = sum_part IND[part, gl] * stats_g[part, j]
            nc.tensor.matmul(ps_g[:], IND[:], stats_g[:], start=True, stop=True)
            gs8 = pl.tile([8, 8], F32, name="gs8")
            nc.vector.tensor_copy(gs8[:], ps_g[:])
            # per-group mean/rstd on 8 partitions x 4 chunks
            invN = 1.0 / float(NTOT)
            mean8 = pl.tile([8, 4], F32, name="mean8")
            var8 = pl.tile([8, 4], F32, name="var8")
            rstd8 = pl.tile([8, 4], F32, name="rstd8")
            eps8 = pl.tile([8, 1], F32, name="eps8")
            nc.vector.memset(eps8[:], EPS)
            nc.vector.tensor_scalar_mul(mean8[:], gs8[:, 0:4], invN)
            nc.vector.tensor_scalar_mul(var8[:], gs8[:, 4:8], invN)
            nc.vector.tensor_tensor(rstd8[:], mean8[:], mean8[:], op=ALU.mult)
            nc.vector.tensor_tensor(var8[:], var8[:], rstd8[:], op=ALU.subtract)
            nc.scalar.activation(var8[:], var8[:], AF.Sqrt, bias=eps8[:])
            nc.vector.reciprocal(rstd8[:], var8[:])
            # pack [rstd | mean] and broadcast groups -> 128 partitions via PE
            rm8 = pl.tile([8, 8], F32, name="rm8")
            nc.vector.tensor_copy(rm8[:, 0:4], rstd8[:])
            nc.vector.tensor_copy(rm8[:, 4:8], mean8[:])
            ps_e = psp.tile([128, 8], F32, name="ps_e", tag="ps_d", bufs=1)
            nc.tensor.matmul(ps_e[:], INDT[:], rm8[:], start=True, stop=True)
            # sc = gamma * rstd; bc = beta - mean * sc   (all chunks at once)
            sc4 = pl.tile([128, 4], F32, name="sc4")
            bc4 = pl.tile([128, 4], F32, name="bc4")
            nc.vector.tensor_tensor(sc4[:], gam4[:], ps_e[:, 0:4], op=ALU.mult)
            nc.vector.tensor_tensor(bc4[:], ps_e[:, 4:8], sc4[:], op=ALU.mult)
            nc.vector.tensor_tensor(bc4[:], bet4[:], bc4[:], op=ALU.subtract)

            if DEBUG:
                dbg_t = pl.tile([128, 64], F32, name="dbg_t")
                nc.vector.memset(dbg_t[:], 0.0)
                nc.vector.tensor_copy(dbg_t[:, 0:8], stats_g[:])
                nc.vector.tensor_copy(dbg_t[0:8, 8:16], gs8[:])
                nc.vector.tensor_copy(dbg_t[0:8, 16:20], mean8[:])
                nc.vector.tensor_copy(dbg_t[0:8, 20:24], var8[:])
                nc.vector.tensor_copy(dbg_t[0:8, 24:28], rstd8[:])
                nc.vector.tensor_copy(dbg_t[:, 28:36], ps_e[:])
                for p in range(PCH):
                    nc.vector.tensor_copy(dbg_t[:, 36 + p : 37 + p], sc_p[p][:])
                    nc.vector.tensor_copy(dbg_t[:, 40 + p : 41 + p], bc_p[p][:])
                    nc.vector.tensor_copy(dbg_t[:, 44 + p : 45 + p], sum_t[p][:])
                    nc.vector.tensor_copy(dbg_t[:, 48 + p : 49 + p], ssq_t[p][:])
                nc.vector.tensor_copy(dbg_t[:, 52:60], IND[:])
                nc.sync.dma_start(dbg_d[:, :], dbg_t[:])

            # ---- pass 2: re-stream x, normalize -> XN2 fp8. n-outer order so
            # the first K^T matmuls (which need all 4 chunks of one n-slice)
            # can start after ~1/8 of the pass ----
            xs2_t = []
            for p in range(PCH):
                xs2 = pl.tile([128, S], F32, name="xs2", tag="xs2", bufs=4)
                nc.sync.dma_start(xs2[:], x_in[p * 128 : (p + 1) * 128, :])
                xs2_t.append(xs2)
            for n in range(NQB):
                for p in range(PCH):
                    nsl = slice(n * QB, (n + 1) * QB)
                    dst = XN2[p // 2][:, p % 2, nsl]
                    if (p + n) % 2 == 0:
                        nc.vector.tensor_scalar(
                            dst, xs2_t[p][:, nsl], sc4[:, p : p + 1], bc4[:, p : p + 1],
                            op0=ALU.mult, op1=ALU.add,
                        )
                    else:
                        nc.scalar.activation(
                            dst, xs2_t[p][:, nsl], AF.Identity,
                            bias=bc4[:, p : p + 1], scale=sc4[:, p : p + 1],
                        )

            prolog_cm.__exit__(None, None, None)

            # ---- main-loop pool ----
            mainloop_cm = tc.tile_pool(name="mainloop", bufs=1)
            ml = mainloop_cm.__enter__()

            # ---- K^T (bias via ACT) and V via DoubleRow on XN2 pairs ----
            DR = mybir.MatmulPerfMode.DoubleRow
            for n in range(NQB):
                for m in range(PCH):
                    ps_k = psp.tile([128, QB], F32, name="ps_k", tag="ps_s", bufs=2)
                    for cc in range(2):
                        nc.tensor.matmul(
                            ps_k[:],
                            W2["wk"][cc][:, :, m * 128 : (m + 1) * 128],
                            XN2[cc][:, :, n * QB : (n + 1) * QB],
                            perf_mode=DR,
                            start=(cc == 0),
                            stop=(cc == 1),
                        )
                    nc.scalar.activation(
                        KT2[m // 2][:, m % 2, n * QB : (n + 1) * QB],
                        ps_k[:],
                        AF.Identity,
                        bias=bk_p[m][:],
                    )
            for j in range(KCH):
                ps_v = psp.tile([128, C], F32, name="ps_v", tag="ps_s", bufs=2)
                for cc in range(2):
                    nc.tensor.matmul(
                        ps_v[:],
                        XN2[cc][:, :, j * 128 : (j + 1) * 128],
                        W2["wv"][cc][:, :, :],
                        perf_mode=DR,
                        start=(cc == 0),
                        stop=(cc == 1),
                    )
                nc.vector.tensor_copy(V2[j // 2][:, j % 2, :], ps_v[:])

            # ---- attention main loop over q-blocks ----
            def emit_qt(qb, m, QT):
                ps_q = psp.tile([128, QB], F32, name="ps_q", tag="ps_q", bufs=1)
                for cc in range(2):
                    nc.tensor.matmul(
                        ps_q[:],
                        W2["wq"][cc][:, :, m * 128 : (m + 1) * 128],
                        XN2[cc][:, :, qb * QB : (qb + 1) * QB],
                        perf_mode=mybir.MatmulPerfMode.DoubleRow,
                        start=(cc == 0),
                        stop=(cc == 1),
                    )
                nc.scalar.activation(
                    QT[m // 2][:, m % 2, :], ps_q[:], AF.Identity, bias=bq_p[m][:]
                )

            def make_qt():
                return [
                    ml.tile([128, 2, QB], FP8, name=f"QT{cc}", tag=f"QT{cc}", bufs=2)
                    for cc in range(2)
                ]

            QT_cur = make_qt()
            for m in range(PCH):
                emit_qt(0, m, QT_cur)

            def emit_s(j, QT, P2pair):
                """scores S^T[j] via DoubleRow fp8 -> exp -> P2 half."""
                ps_s = psp.tile([128, QB], F32, name="ps_s", tag="ps_s", bufs=2)
                for cc in range(2):
                    nc.tensor.matmul(
                        ps_s[:],
                        KT2[cc][:, :, j * 128 : (j + 1) * 128],
                        QT[cc][:],
                        perf_mode=mybir.MatmulPerfMode.DoubleRow,
                        start=(cc == 0),
                        stop=(cc == 1),
                    )
                nc.scalar.activation(
                    P2pair[:, j % 2, :], ps_s[:], AF.Exp, scale=SCALE
                )

            NJJ = KCH // 2  # 16 pairs
            for qb in range(NQB):
                QT_next = None
                # software-pipelined over PAIRS: s one pair ahead, then
                # d_jj + PV_jj consume pair jj (DoubleRow fp8)
                ps_dd = psp.tile([1, QB], F32, name="ps_dd", tag="ps_d", bufs=1)
                ps_o = [
                    psp.tile([128, QB], F32, name=f"ps_o{mc}", tag=f"ps_o{mc}", bufs=1)
                    for mc in range(PCH)
                ]

                def make_pair():
                    return ml.tile([128, 2, QB], FP8, name="P2", tag="P2", bufs=4)

                P2s = [None] * NJJ
                P2s[0] = make_pair()
                emit_s(0, QT_cur, P2s[0])
                emit_s(1, QT_cur, P2s[0])
                P2s[1] = make_pair()
                emit_s(2, QT_cur, P2s[1])
                emit_s(3, QT_cur, P2s[1])
                for jj in range(NJJ):
                    if jj + 2 < NJJ:
                        P2s[jj + 2] = make_pair()
                        emit_s(2 * jj + 4, QT_cur, P2s[jj + 2])
                        emit_s(2 * jj + 5, QT_cur, P2s[jj + 2])
                    nc.tensor.matmul(
                        ps_dd[:],
                        ones2[:, :, 0:1],
                        P2s[jj][:],
                        perf_mode=mybir.MatmulPerfMode.DoubleRow,
                        start=(jj == 0),
                        stop=(jj == NJJ - 1),
                    )
                    for mc in range(PCH):
                        nc.tensor.matmul(
                            ps_o[mc][:],
                            V2[jj][:, :, mc * 128 : (mc + 1) * 128],
                            P2s[jj][:],
                            perf_mode=mybir.MatmulPerfMode.DoubleRow,
                            start=(jj == 0),
                            stop=(jj == NJJ - 1),
                        )
                    P2s[jj] = None
                    # interleave next block's q^T generation into the PV stream
                    if jj % 4 == 1 and qb + 1 < NQB:
                        if QT_next is None:
                            QT_next = make_qt()
                        emit_qt(qb + 1, (jj - 1) // 4, QT_next)

                # raw attention sums -> bf16 via ACT (fast, off DVE); the
                # 1/d column scaling commutes with the wp matmul and is
                # applied in the epilogue instead.
                OT2 = [
                    ml.tile([128, 2, QB], FP8, name=f"OT2_{cc}", tag=f"OT2_{cc}", bufs=1)
                    for cc in range(2)
                ]
                for mc in range(PCH):
                    if mc % 2 == 0:
                        nc.scalar.copy(OT2[mc // 2][:, mc % 2, :], ps_o[mc][:])
                    else:
                        nc.vector.tensor_copy(OT2[mc // 2][:, mc % 2, :], ps_o[mc][:])

                # denominators -> r broadcast (fully overlapped with wp MMs):
                # ACT copy psum->sbuf, PE rank-1 f32r broadcast, wide DVE recip
                d_sb = ml.tile([1, QB], mybir.dt.float32r, name="d_sb", tag="d_sb", bufs=2)
                r_bc = ml.tile([128, QB], F32, name="r_bc", tag="r_bc", bufs=2)
                nc.scalar.copy(d_sb[:], ps_dd[:])
                ps_r = psp.tile([128, QB], F32, name="ps_r", tag="ps_q", bufs=1)
                nc.tensor.matmul(ps_r[:], ones_row[:], d_sb[:], start=True, stop=True)
                nc.vector.reciprocal(r_bc[:], ps_r[:])

                # project by wp; epilogue: scale by r, add bp' and residual x
                q0 = qb * QB
                for m in range(PCH):
                    ps_f = psp.tile([128, QB], F32, name="ps_f", tag=f"ps_o{m}", bufs=1)
                    for cc in range(2):
                        nc.tensor.matmul(
                            ps_f[:],
                            Wp2[cc][:, :, m * 128 : (m + 1) * 128],
                            OT2[cc][:],
                            perf_mode=mybir.MatmulPerfMode.DoubleRow,
                            start=(cc == 0),
                            stop=(cc == 1),
                        )
                    xr = ml.tile([128, QB], F32, name="xr", tag="xr", bufs=4)
                    nc.sync.dma_start(xr[:], x_in[m * 128 : (m + 1) * 128, q0 : q0 + QB])
                    on_ = ml.tile([128, QB], F32, name="on", tag="on", bufs=4)
                    os_ = ml.tile([128, QB], F32, name="os", tag="os", bufs=4)
                    nc.vector.tensor_tensor(on_[:], ps_f[:], r_bc[:], op=ALU.mult)
                    nc.vector.scalar_tensor_tensor(
                        os_[:], on_[:], bpp_p[m][:], xr[:], op0=ALU.add, op1=ALU.add
                    )
                    nc.sync.dma_start(
                        out_d[m * 128 : (m + 1) * 128, q0 : q0 + QB], os_[:]
                    )
                if QT_next is not None:
                    QT_cur = QT_next

            mainloop_cm.__exit__(None, None, None)

    nc.compile()
    return nc


def _get_nc():
    if "nc" not in _NC_CACHE:
        _NC_CACHE["nc"] = build_nc()
    return _NC_CACHE["nc"]


def kernel(x, gamma, beta, wq, bq, wk, bk, wv, bv, wp, bp, **_unused):
    x = np.asarray(x, np.float32)
    b, c, t, h, w = x.shape
    assert (b, c, t, h, w) == (1, C, 8, 64, 64)
    nc = _get_nc()

    shared = {
        "gamma": np.ascontiguousarray(np.asarray(gamma, np.float32)),
        "beta": np.ascontiguousarray(np.asarray(beta, np.float32)),
        "wq": np.ascontiguousarray(np.asarray(wq, np.float32)),
        "bq": np.ascontiguousarray(np.asarray(bq, np.float32)),
        "wk": np.ascontiguousarray(np.asarray(wk, np.float32)),
        "bk": np.ascontiguousarray(np.asarray(bk, np.float32)),
        "wv": np.ascontiguousarray(np.asarray(wv, np.float32)),
        "bv": np.ascontiguousarray(np.asarray(bv, np.float32)),
        "wp": np.ascontiguousarray(np.asarray(wp, np.float32)),
        "bp": np.ascontiguousarray(np.asarray(bp, np.float32)),
    }
    in_maps = []
    for ti in range(t):
        frame = np.ascontiguousarray(x[0, :, ti, :, :].reshape(C, S))
        in_maps.append({"x": frame, **shared})

    res = run_bass_kernel_spmd(nc, in_maps, core_ids=list(range(N_CORES)))

    out = np.empty((1, C, t, h, w), np.float32)
    for ti in range(t):
        out[0, :, ti, :, :] = res.results[ti]["out"].reshape(C, h, w)
    return out


# revision 23
# speedup vs baseline: 1.1127x; 1.0512x over previous
"""AttnBlock3D Trainium2 kernel (8-core frame-parallel).

Math (per reference):
  hn = GroupNorm32(x) * gamma + beta          # stats over (c/32, t, h, w) -> global over frames
  q/k/v = hn @ w{q,k,v} + b{q,k,v}            # per-frame, per-position linear over channels
  attn  = softmax(q @ k.T / sqrt(c))          # per frame, positions hw=4096
  o     = attn @ v @ wp + bp
  out   = x + o

Distribution: one frame (b*t = 8) per NeuronCore. GroupNorm stats need a
cross-frame reduction: each core computes per-channel sum/sumsq over its
frame, a 4KB AllReduce combines them, then everything else is local.
Group-stat math (16-channel segment sums, group->channel broadcast) runs on
the PE via tiny indicator-matrix matmuls to avoid slow 1-partition DVE ops.

On-chip layouts (SBUF partitions x free):
  XN  [c=512 (4x128), pos=4096] bf16     normalized activations, transposed
  KT  [c_out (4x128), pos=4096] bf16     k transposed
  V   [pos (32x128), c_out=512] bf16     v natural
  per q-block (512 positions), flash-pipelined over k-chunks j:
    S_j psum [128, 512] -> exp -> P_j bf16 (few rotating slots)
    d  psum [1,512] += ones.T @ P_j   (softmax denominators on PE)
    O  psum [128c, 512] += V_j.T @ P_j
    r = recip(bcast(d)); OT = O * r; out = wp.T @ OT + bp' + x (f32 residual)
  Biases: q,k as per-partition ACT bias; v,p folded: bp' = wp.T @ bv + bp.
"""

import sys

sys.path.insert(0, "/opt/trn_rl_repo")

import numpy as np

import concourse.bacc as bacc
import concourse.bass as bass
import concourse.mybir as mybir
import concourse.tile as tile
from concourse.bass_utils import run_bass_kernel_spmd

N_CORES = 8
C = 512  # channels
S = 4096  # positions per frame (h*w)
G = 32  # groups
CPG = C // G  # 16 channels per group
PCH = C // 128  # 4 channel chunks of 128 partitions
KCH = S // 128  # 32 position chunks of 128
QB = 512  # q-block size
NQB = S // QB  # 8 q blocks
NTOT = CPG * 8 * S  # group-norm element count per group (global over 8 frames)
EPS = 1e-6
SCALE = float(C) ** -0.5

F32 = mybir.dt.float32
BF16 = mybir.dt.bfloat16
AF = mybir.ActivationFunctionType
ALU = mybir.AluOpType
AX = mybir.AxisListType

_NC_CACHE = {}
DEBUG = False


def xs2_dst8(nc, dst):
    return dst


def build_nc():
    nc = bacc.Bacc("TRN2", target_bir_lowering=False, debug=False, num_devices=N_CORES)

    x_in = nc.dram_tensor("x", [C, S], F32, kind="ExternalInput")
    gamma_in = nc.dram_tensor("gamma", [C], F32, kind="ExternalInput")
    beta_in = nc.dram_tensor("beta", [C], F32, kind="ExternalInput")
    w_in = {}
    b_in = {}
    for nm in ("wq", "wk", "wv", "wp"):
        w_in[nm] = nc.dram_tensor(nm, [C, C], F32, kind="ExternalInput")
    for nm in ("bq", "bk", "bv", "bp"):
        b_in[nm] = nc.dram_tensor(nm, [C], F32, kind="ExternalInput")
    out_d = nc.dram_tensor("out", [C, S], F32, kind="ExternalOutput")
    dbg_d = nc.dram_tensor("dbg", [128, 64], F32, kind="ExternalOutput") if DEBUG else None

    with tile.TileContext(nc) as tc:
        with (
            tc.tile_pool(name="persist", bufs=1) as pp,
            tc.tile_pool(name="psum", bufs=1, space="PSUM") as psp,
            tc.tile_pool(name="dram", bufs=1, space="DRAM") as dram,
        ):
            # ---- persistent SBUF ----
            FP8 = mybir.dt.float8e4
            # fp8 pair tiles for DoubleRow matmuls: [:, parity, :]
            XN2 = [pp.tile([128, 2, S], FP8, name=f"XN2_{cc}") for cc in range(2)]
            KT2 = [pp.tile([128, 2, S], FP8, name=f"KT2_{cc}") for cc in range(2)]
            V2 = [pp.tile([128, 2, C], FP8, name=f"V2_{jj}") for jj in range(KCH // 2)]
            W2 = {
                nm: [pp.tile([128, 2, C], FP8, name=f"{nm}2_{cc}") for cc in range(2)]
                for nm in ("wq", "wk", "wv")
            }
            Wp = [pp.tile([128, C], BF16, name=f"wp_{p}") for p in range(PCH)]
            Wp2 = [pp.tile([128, 2, C], FP8, name=f"wp2_{cc}") for cc in range(2)]
            bq_p = [pp.tile([128, 1], F32, name=f"bqp{p}") for p in range(PCH)]
            bk_p = [pp.tile([128, 1], F32, name=f"bkp{p}") for p in range(PCH)]
            bv_bf = [pp.tile([128, 1], BF16, name=f"bvb{p}") for p in range(PCH)]
            bpp_p = [pp.tile([128, 1], F32, name=f"bppp{p}") for p in range(PCH)]
            sc_p = [pp.tile([128, 1], F32, name=f"scp{p}") for p in range(PCH)]
            bc_p = [pp.tile([128, 1], F32, name=f"bcp{p}") for p in range(PCH)]
            ones2 = pp.tile([128, 2, 16], FP8, name="ones2")
            nc.vector.memset(ones2[:], 1.0)

            F32R = mybir.dt.float32r
            ones_row_f = pp.tile([1, 128], F32, name="ones_row_f")
            ones_row = pp.tile([1, 128], F32R, name="ones_row")
            nc.vector.memset(ones_row_f[:], 1.0)
            nc.vector.tensor_copy(ones_row[:], ones_row_f[:])

            # ---- prologue pool (released before attention main loop) ----
            prolog_cm = tc.tile_pool(name="prolog", bufs=1)
            pl = prolog_cm.__enter__()

            # ---- pass 1 first: stream x (critical path), sum & sumsq ----
            sum_t = [pl.tile([128, 1], F32, name=f"sum{p}") for p in range(PCH)]
            ssq_t = [pl.tile([128, 1], F32, name=f"ssq{p}") for p in range(PCH)]
            for p in range(PCH):
                xs = pl.tile([128, S], F32, name="xs", tag="xstream", bufs=2)
                nc.sync.dma_start(xs[:], x_in[p * 128 : (p + 1) * 128, :])
                nc.vector.reduce_sum(sum_t[p][:], xs[:], axis=AX.X)
                junk = pl.tile([128, S], BF16, name="junk", tag="junk", bufs=1)
                nc.scalar.activation(junk[:], xs[:], AF.Square, accum_out=ssq_t[p][:])

            # pack stats -> [128, 8]: cols 0-3 sums, 4-7 sumsq; AllReduce
            stats8 = pl.tile([128, 8], F32, name="stats8")
            for p in range(PCH):
                nc.vector.tensor_copy(stats8[:, p : p + 1], sum_t[p][:])
                nc.vector.tensor_copy(stats8[:, 4 + p : 5 + p], ssq_t[p][:])
            cc_in = dram.tile([128, 8], F32, name="cc_in")
            cc_out = dram.tile([128, 8], F32, name="cc_out", addr_space="Shared")
            nc.sync.dma_start(cc_in[:], stats8[:])
            nc.gpsimd.collective_compute(
                "AllReduce",
                ALU.add,
                replica_groups=[list(range(N_CORES))],
                ins=[cc_in.opt()],
                outs=[cc_out.opt()],
            )

            # ---- weight load + cast (streamed); wp first (bp' needs it) ----
            for nm in ("wp", "wq", "wk", "wv"):
                for p in range(PCH):
                    wstg = pl.tile([128, C], F32, name="wstg", tag="wstg", bufs=2)
                    nc.sync.dma_start(wstg[:], w_in[nm][p * 128 : (p + 1) * 128, :])
                    if nm == "wp":
                        nc.vector.tensor_copy(Wp[p][:], wstg[:])
                        nc.vector.tensor_copy(Wp2[p // 2][:, p % 2, :], wstg[:])
                    else:
                        nc.vector.tensor_copy(W2[nm][p // 2][:, p % 2, :], wstg[:])

            # ---- small loads (off critical path) ----
            for p in range(PCH):
                nc.sync.dma_start(bq_p[p][:], b_in["bq"][p * 128 : (p + 1) * 128, None])
                nc.sync.dma_start(bk_p[p][:], b_in["bk"][p * 128 : (p + 1) * 128, None])
            bv_st = [pl.tile([128, 1], F32, name=f"bvst{p}") for p in range(PCH)]
            bp_p = [pl.tile([128, 1], F32, name=f"bpst{p}") for p in range(PCH)]
            gam4 = pl.tile([128, 4], F32, name="gam4")
            bet4 = pl.tile([128, 4], F32, name="bet4")
            # channel c=(part,p): DRAM idx 128p+part -> [128 part, 4 p] strided
            nc.sync.dma_start(gam4[:], gamma_in[:].rearrange("(p a) -> a p", p=4, a=128))
            nc.sync.dma_start(bet4[:], beta_in[:].rearrange("(p a) -> a p", p=4, a=128))
            for p in range(PCH):
                sl = slice(p * 128, (p + 1) * 128)
                nc.sync.dma_start(bv_st[p][:], b_in["bv"][sl, None])
                nc.sync.dma_start(bp_p[p][:], b_in["bp"][sl, None])
                nc.vector.tensor_copy(bv_bf[p][:], bv_st[p][:])

            # indicator matrices for group-segment sums / broadcasts
            ind_np = np.zeros((128, 8), np.float32)  # [part, gl] = part//16==gl
            for gl in range(8):
                ind_np[16 * gl : 16 * (gl + 1), gl] = 1.0
            ind_d = nc.inline_tensor(ind_np, name="ind_const")
            indt_d = nc.inline_tensor(np.ascontiguousarray(ind_np.T), name="indt_const")
            IND = pl.tile([128, 8], F32, name="IND")
            INDT = pl.tile([8, 128], F32, name="INDT")
            nc.sync.dma_start(IND[:], ind_d[:, :])
            nc.sync.dma_start(INDT[:], indt_d[:, :])

            # ---- bp' = wp.T @ bv + bp via N=1 matmuls ----
            for m in range(PCH):
                ps_bp = psp.tile([128, 1], F32, name="ps_bp", tag="ps_d", bufs=1)
                for ci in range(PCH):
                    nc.tensor.matmul(
                        ps_bp[:],
                        Wp[ci][:, m * 128 : (m + 1) * 128],
                        bv_bf[ci][:],
                        start=(ci == 0),
                        stop=(ci == PCH - 1),
                    )
                nc.vector.tensor_tensor(bpp_p[m][:], ps_bp[:], bp_p[m][:], op=ALU.add)

            # ---- post-collective: group stats on PE ----
            stats_g = pl.tile([128, 8], F32, name="stats_g")
            nc.sync.dma_start(stats_g[:], cc_out[:])
            ps_g = psp.tile([8, 8], F32, name="ps_g", tag="ps_d", bufs=1)
            # out[gl, j] = sum_part IND[part, gl] * stats_g[part, j]
            nc.tensor.matmul(ps_g[:], IND[:], stats_g[:], start=True, stop=True)
            gs8 = pl.tile([8, 8], F32, name="gs8")
            nc.vector.tensor_copy(gs8[:], ps_g[:])
            # per-group mean/rstd on 8 partitions x 4 chunks
            invN = 1.0 / float(NTOT)
            mean8 = pl.tile([8, 4], F32, name="mean8")
            var8 = pl.tile([8, 4], F32, name="var8")
            rstd8 = pl.tile([8, 4], F32, name="rstd8")
            eps8 = pl.tile([8, 1], F32, name="eps8")
            nc.vector.memset(eps8[:], EPS)
            nc.vector.tensor_scalar_mul(mean8[:], gs8[:, 0:4], invN)
            nc.vector.tensor_scalar_mul(var8[:], gs8[:, 4:8], invN)
            nc.vector.tensor_tensor(rstd8[:], mean8[:], mean8[:], op=ALU.mult)
            nc.vector.tensor_tensor(var8[:], var8[:], rstd8[:], op=ALU.subtract)
            nc.scalar.activation(var8[:], var8[:], AF.Sqrt, bias=eps8[:])
            nc.vector.reciprocal(rstd8[:], var8[:])
            # pack [rstd | mean] and broadcast groups -> 128 partitions via PE
            rm8 = pl.tile([8, 8], F32, name="rm8")
            nc.vector.tensor_copy(rm8[:, 0:4], rstd8[:])
            nc.vector.tensor_copy(rm8[:, 4:8], mean8[:])
            ps_e = psp.tile([128, 8], F32, name="ps_e", tag="ps_d", bufs=1)
            nc.tensor.matmul(ps_e[:], INDT[:], rm8[:], start=True, stop=True)
            # sc = gamma * rstd; bc = beta - mean * sc   (all chunks at once)
            sc4 = pl.tile([128, 4], F32, name="sc4")
            bc4 = pl.tile([128, 4], F32, name="bc4")
            nc.vector.tensor_tensor(sc4[:], gam4[:], ps_e[:, 0:4], op=ALU.mult)
            nc.vector.tensor_tensor(bc4[:], ps_e[:, 4:8], sc4[:], op=ALU.mult)
            nc.vector.tensor_tensor(bc4[:], bet4[:], bc4[:], op=ALU.subtract)

            if DEBUG:
                dbg_t = pl.tile([128, 64], F32, name="dbg_t")
                nc.vector.memset(dbg_t[:], 0.0)
                nc.vector.tensor_copy(dbg_t[:, 0:8], stats_g[:])
                nc.vector.tensor_copy(dbg_t[0:8, 8:16], gs8[:])
                nc.vector.tensor_copy(dbg_t[0:8, 16:20], mean8[:])
                nc.vector.tensor_copy(dbg_t[0:8, 20:24], var8[:])
                nc.vector.tensor_copy(dbg_t[0:8, 24:28], rstd8[:])
                nc.vector.tensor_copy(dbg_t[:, 28:36], ps_e[:])
                for p in range(PCH):
                    nc.vector.tensor_copy(dbg_t[:, 36 + p : 37 + p], sc_p[p][:])
                    nc.vector.tensor_copy(dbg_t[:, 40 + p : 41 + p], bc_p[p][:])
                    nc.vector.tensor_copy(dbg_t[:, 44 + p : 45 + p], sum_t[p][:])
                    nc.vector.tensor_copy(dbg_t[:, 48 + p : 49 + p], ssq_t[p][:])
                nc.vector.tensor_copy(dbg_t[:, 52:60], IND[:])
                nc.sync.dma_start(dbg_d[:, :], dbg_t[:])

            # ---- pass 2: re-stream x, normalize -> XN2 fp8. n-outer order so
            # the first K^T matmuls (which need all 4 chunks of one n-slice)
            # can start after ~1/8 of the pass ----
            xs2_t = []
            for p in range(PCH):
                xs2 = pl.tile([128, S], F32, name="xs2", tag="xs2", bufs=4)
                nc.sync.dma_start(xs2[:], x_in[p * 128 : (p + 1) * 128, :])
                xs2_t.append(xs2)
            for n in range(NQB):
                for p in range(PCH):
                    nsl = slice(n * QB, (n + 1) * QB)
                    dst = XN2[p // 2][:, p % 2, nsl]
                    if (p + n) % 2 == 0:
                        nc.vector.tensor_scalar(
                            dst, xs2_t[p][:, nsl], sc4[:, p : p + 1], bc4[:, p : p + 1],
                            op0=ALU.mult, op1=ALU.add,
                        )
                    else:
                        nc.scalar.activation(
                            dst, xs2_t[p][:, nsl], AF.Identity,
                            bias=bc4[:, p : p + 1], scale=sc4[:, p : p + 1],
                        )

            prolog_cm.__exit__(None, None, None)

            # ---- main-loop pool ----
            mainloop_cm = tc.tile_pool(name="mainloop", bufs=1)
            ml = mainloop_cm.__enter__()

            # ---- K^T (bias via ACT) and V via DoubleRow on XN2 pairs ----
            DR = mybir.MatmulPerfMode.DoubleRow
            for n in range(NQB):
                for m in range(PCH):
                    ps_k = psp.tile([128, QB], F32, name="ps_k", tag="ps_s", bufs=2)
                    for cc in range(2):
                        nc.tensor.matmul(
                            ps_k[:],
                            W2["wk"][cc][:, :, m * 128 : (m + 1) * 128],
                            XN2[cc][:, :, n * QB : (n + 1) * QB],
                            perf_mode=DR,
                            start=(cc == 0),
                            stop=(cc == 1),
                        )
                    nc.scalar.activation(
                        KT2[m // 2][:, m % 2, n * QB : (n + 1) * QB],
                        ps_k[:],
                        AF.Identity,
                        bias=bk_p[m][:],
                    )
            for j in range(KCH):
                ps_v = psp.tile([128, C], F32, name="ps_v", tag="ps_s", bufs=2)
                for cc in range(2):
                    nc.tensor.matmul(
                        ps_v[:],
                        XN2[cc][:, :, j * 128 : (j + 1) * 128],
                        W2["wv"][cc][:, :, :],
                        perf_mode=DR,
                        start=(cc == 0),
                        stop=(cc == 1),
                    )
                nc.vector.tensor_copy(V2[j // 2][:, j % 2, :], ps_v[:])

            # ---- attention main loop over q-blocks ----
            def emit_qt(qb, m, QT):
                ps_q = psp.tile([128, QB], F32, name="ps_q", tag="ps_q", bufs=1)
                for cc in range(2):
                    nc.tensor.matmul(
                        ps_q[:],
                        W2["wq"][cc][:, :, m * 128 : (m + 1) * 128],
                        XN2[cc][:, :, qb * QB : (qb + 1) * QB],
                        perf_mode=mybir.MatmulPerfMode.DoubleRow,
                        start=(cc == 0),
                        stop=(cc == 1),
                    )
                nc.scalar.activation(
                    QT[m // 2][:, m % 2, :], ps_q[:], AF.Identity, bias=bq_p[m][:]
                )

            def make_qt():
                return [
                    ml.tile([128, 2, QB], FP8, name=f"QT{cc}", tag=f"QT{cc}", bufs=2)
                    for cc in range(2)
                ]

            QT_cur = make_qt()
            for m in range(PCH):
                emit_qt(0, m, QT_cur)

            def emit_s(j, QT, P2pair):
                """scores S^T[j] via DoubleRow fp8 -> exp -> P2 half."""
                ps_s = psp.tile([128, QB], F32, name="ps_s", tag="ps_s", bufs=2)
                for cc in range(2):
                    nc.tensor.matmul(
                        ps_s[:],
                        KT2[cc][:, :, j * 128 : (j + 1) * 128],
                        QT[cc][:],
                        perf_mode=mybir.MatmulPerfMode.DoubleRow,
                        start=(cc == 0),
                        stop=(cc == 1),
                    )
                nc.scalar.activation(
                    P2pair[:, j % 2, :], ps_s[:], AF.Exp, scale=SCALE
                )

            NJJ = KCH // 2  # 16 pairs
            for qb in range(NQB):
                QT_next = None
                # software-pipelined over PAIRS: s one pair ahead, then
                # d_jj + PV_jj consume pair jj (DoubleRow fp8)
                ps_dd = psp.tile([1, QB], F32, name="ps_dd", tag="ps_d", bufs=1)
                ps_o = [
                    psp.tile([128, QB], F32, name=f"ps_o{mc}", tag=f"ps_o{mc}", bufs=1)
                    for mc in range(PCH)
                ]

                def make_pair():
                    return ml.tile([128, 2, QB], FP8, name="P2", tag="P2", bufs=4)

                P2s = [None] * NJJ
                P2s[0] = make_pair()
                emit_s(0, QT_cur, P2s[0])
                emit_s(1, QT_cur, P2s[0])
                P2s[1] = make_pair()
                emit_s(2, QT_cur, P2s[1])
                emit_s(3, QT_cur, P2s[1])
                for jj in range(NJJ):
                    if jj + 2 < NJJ:
                        P2s[jj + 2] = make_pair()
                        emit_s(2 * jj + 4, QT_cur, P2s[jj + 2])
                        emit_s(2 * jj + 5, QT_cur, P2s[jj + 2])
                    nc.tensor.matmul(
                        ps_dd[:],
                        ones2[:, :, 0:1],
                        P2s[jj][:],
                        perf_mode=mybir.MatmulPerfMode.DoubleRow,
                        start=(jj == 0),
                        stop=(jj == NJJ - 1),
                    )
                    for mc in range(PCH):
                        nc.tensor.matmul(
                            ps_o[mc][:],
                            V2[jj][:, :, mc * 128 : (mc + 1) * 128],
                            P2s[jj][:],
                            perf_mode=mybir.MatmulPerfMode.DoubleRow,
                            start=(jj == 0),
                            stop=(jj == NJJ - 1),
                        )
                    P2s[jj] = None
                    # interleave next block's q^T generation into the PV stream
                    if jj % 4 == 1 and qb + 1 < NQB:
                        if QT_next is None:
                            QT_next = make_qt()
                        emit_qt(qb + 1, (jj - 1) // 4, QT_next)

                # raw attention sums -> bf16 via ACT (fast, off DVE); the
                # 1/d column scaling commutes with the wp matmul and is
                # applied in the epilogue instead.
                # denominators -> r broadcast first (overlaps tail of PV):
                # ACT copy psum->sbuf, PE rank-1 f32r broadcast, fast DVE recip
                d_sb = ml.tile([1, QB], mybir.dt.float32r, name="d_sb", tag="d_sb", bufs=2)
                r_bc = ml.tile([128, QB], F32, name="r_bc", tag="r_bc", bufs=2)
                nc.scalar.copy(d_sb[:], ps_dd[:])
                ps_r = psp.tile([128, QB], F32, name="ps_r", tag="ps_q", bufs=1)
                nc.tensor.matmul(ps_r[:], ones_row[:], d_sb[:], start=True, stop=True)
                nc.vector.reciprocal_approx_fast(r_bc[:], ps_r[:])

                OT2 = [
                    ml.tile([128, 2, QB], FP8, name=f"OT2_{cc}", tag=f"OT2_{cc}", bufs=1)
                    for cc in range(2)
                ]
                for mc in range(PCH):
                    if mc % 2 == 0:
                        nc.scalar.copy(OT2[mc // 2][:, mc % 2, :], ps_o[mc][:])
                    else:
                        nc.vector.tensor_copy(OT2[mc // 2][:, mc % 2, :], ps_o[mc][:])

                # project by wp; epilogue: scale by r, add bp' and residual x
                q0 = qb * QB
                for m in range(PCH):
                    ps_f = psp.tile([128, QB], F32, name="ps_f", tag=f"ps_o{m}", bufs=1)
                    for cc in range(2):
                        nc.tensor.matmul(
                            ps_f[:],
                            Wp2[cc][:, :, m * 128 : (m + 1) * 128],
                            OT2[cc][:],
                            perf_mode=mybir.MatmulPerfMode.DoubleRow,
                            start=(cc == 0),
                            stop=(cc == 1),
                        )
                    xr = ml.tile([128, QB], F32, name="xr", tag="xr", bufs=4)
                    nc.sync.dma_start(xr[:], x_in[m * 128 : (m + 1) * 128, q0 : q0 + QB])
                    on_ = ml.tile([128, QB], F32, name="on", tag="on", bufs=4)
                    os_ = ml.tile([128, QB], F32, name="os", tag="os", bufs=4)
                    nc.vector.tensor_tensor(on_[:], ps_f[:], r_bc[:], op=ALU.mult)
                    nc.vector.scalar_tensor_tensor(
                        os_[:], on_[:], bpp_p[m][:], xr[:], op0=ALU.add, op1=ALU.add
                    )
                    nc.sync.dma_start(
                        out_d[m * 128 : (m + 1) * 128, q0 : q0 + QB], os_[:]
                    )
                if QT_next is not None:
                    QT_cur = QT_next

            mainloop_cm.__exit__(None, None, None)

    nc.compile()
    return nc


def _get_nc():
    if "nc" not in _NC_CACHE:
        _NC_CACHE["nc"] = build_nc()
    return _NC_CACHE["nc"]


def kernel(x, gamma, beta, wq, bq, wk, bk, wv, bv, wp, bp, **_unused):
    x = np.asarray(x, np.float32)
    b, c, t, h, w = x.shape
    assert (b, c, t, h, w) == (1, C, 8, 64, 64)
    nc = _get_nc()

    shared = {
        "gamma": np.ascontiguousarray(np.asarray(gamma, np.float32)),
        "beta": np.ascontiguousarray(np.asarray(beta, np.float32)),
        "wq": np.ascontiguousarray(np.asarray(wq, np.float32)),
        "bq": np.ascontiguousarray(np.asarray(bq, np.float32)),
        "wk": np.ascontiguousarray(np.asarray(wk, np.float32)),
        "bk": np.ascontiguousarray(np.asarray(bk, np.float32)),
        "wv": np.ascontiguousarray(np.asarray(wv, np.float32)),
        "bv": np.ascontiguousarray(np.asarray(bv, np.float32)),
        "wp": np.ascontiguousarray(np.asarray(wp, np.float32)),
        "bp": np.ascontiguousarray(np.asarray(bp, np.float32)),
    }
    in_maps = []
    for ti in range(t):
        frame = np.ascontiguousarray(x[0, :, ti, :, :].reshape(C, S))
        in_maps.append({"x": frame, **shared})

    res = run_bass_kernel_spmd(nc, in_maps, core_ids=list(range(N_CORES)))

    out = np.empty((1, C, t, h, w), np.float32)
    for ti in range(t):
        out[0, :, ti, :, :] = res.results[ti]["out"].reshape(C, h, w)
    return out
